# revision 25
# baseline (speedup 1.0000x reference)
"""Trainium2 Bass kernel for windowed 3D attention (nn_Attention_12927851561046).

512 windows of 343-token, 4-head, 32-dim-per-head attention over d=128.
Pure data parallel: 64 windows per core across 8 NeuronCores.

Layout strategy (per window):
  XT (d=128 partitions, 343 tokens free) bf16
  qT/kT = w^T@XT  -> psum -> cast to bf16 sbuf (128=4h*32dh, 343)
  v    = XT^T@wv  -> psum (t-chunks, 128) -> cast bf16 sbuf (128, 3*128)
  simT chunks (j on partitions, i free), 2 heads per psum tile (128, 686)
  exp on ACT (psum->sbuf bf16), *expbias on DVE/GPSIMD (bf16 2x)
  attnout^T + replicated rowsums via ones-matmul (col-tiled, head-packed)
  1/rowsum via DVE reciprocal_approx_fast, normalize+cast on DVE
  final = anrm^T @ w_out -> psum -> copy -> DMA out
"""

import os
import sys
from contextlib import ExitStack

import numpy as np

sys.path.insert(0, "/opt/trn_rl_repo")

import ml_dtypes  # noqa: E402

import concourse.bass as bass  # noqa: E402
import concourse.tile as tile  # noqa: E402
from concourse.tile import add_dep_helper  # noqa: E402
from concourse import bacc, mybir  # noqa: E402
from concourse import bass_utils  # noqa: E402

BF16 = mybir.dt.bfloat16
F32 = mybir.dt.float32

NW = 64          # windows per core
N = 343          # tokens per window
D = 128
H = 4
DH = 32
NP = 384         # padded tokens (zeros beyond 343)
JOFF = [0, 128, 256]

# bisection toggles
NO_GPSIMD = not bool(int(os.environ.get("K_GPSIMD", "0")))
NO_RECIP_FAST = bool(int(os.environ.get("K_NO_RECIP_FAST", "0")))
NO_TILEPOS_SIM = bool(int(os.environ.get("K_NO_TILEPOS_SIM", "0")))
NO_TILEPOS_AO = bool(int(os.environ.get("K_NO_TILEPOS_AO", "0")))
STAGE = int(os.environ.get("K_STAGE", "0"))  # 0=full, 1..4 truncation
H2ONLY = bool(int(os.environ.get("K_H2ONLY", "0")))  # heads use bases {0,32} only (wrong data, mechanism test)
EXP_SPLIT = bool(int(os.environ.get("K_EXP_SPLIT", "1")))  # per-bank exp reads

# module-level knobs (test.py pokes these)
TRACE = False
TRACE_KWARGS = {}

_cache = {}


def _build_kernel():
    nc = bacc.Bacc(
        "TRN2",
        target_bir_lowering=False,
        debug=False,
        enable_asserts=False,
        num_devices=8,
    )
    xt_d = nc.dram_tensor("xt", (NW, D, NP), BF16, kind="ExternalInput").ap()
    wqkv_d = nc.dram_tensor("wqkv", (D, 3 * D), BF16, kind="ExternalInput").ap()
    wout_d = nc.dram_tensor("wout", (D, D), BF16, kind="ExternalInput").ap()
    eb_d = nc.dram_tensor("eb", (D, 3 * H * N), BF16, kind="ExternalInput").ap()
    out_d = nc.dram_tensor("out", (NW, N, D), F32, kind="ExternalOutput").ap()

    with tile.TileContext(nc) as tc:
        with ExitStack() as ctx:
            _body(ctx, tc, out_d, xt_d, wqkv_d, wout_d, eb_d)

    nc.compile()
    return nc


def _chain(insts):
    for a, b in zip(insts[1:], insts[:-1]):
        add_dep_helper(a.ins, b.ins, sync=False, reason="psum accumulation order")


def _body(ctx, tc, out_d, xt_d, wqkv_d, wout_d, eb_d):
    nc = tc.nc

    const = ctx.enter_context(tc.tile_pool(name="const", bufs=1))
    sb = ctx.enter_context(tc.tile_pool(name="sb", bufs=2))
    ps = ctx.enter_context(tc.tile_pool(name="ps", bufs=1, space="PSUM"))

    # constants
    wqkv = const.tile([D, 3 * D], BF16)
    nc.sync.dma_start(wqkv[:], wqkv_d[:])
    wout = const.tile([D, D], BF16)
    nc.sync.dma_start(wout[:], wout_d[:])
    eb = const.tile([D, 3 * H * N], BF16)
    nc.sync.dma_start(eb[:], eb_d[:])
    ones = const.tile([D, D], BF16)
    nc.vector.memset(ones[:], 1.0)

    def tail(w, aop, rsp, rs_last, ao_last, anrm_of):
        """Deferred per-window tail: normalize, out-projection, output DMA."""
        recip = sb.tile([D, N], F32, tag="recip", bufs=3)
        rc = nc.vector.reciprocal_approx_fast(recip[:], rsp[:])
        add_dep_helper(rc.ins, rs_last.ins, sync=True,
                       reason="read rowsums after accumulation closes")
        anrm = sb.tile([D, N], BF16, tag="anrm", bufs=3)
        tt = nc.vector.tensor_mul(anrm[:], aop[:], recip[:])
        add_dep_helper(tt.ins, ao_last.ins, sync=True,
                       reason="read ao after accumulation closes")

        fp = ps.tile([D, 3 * D], F32, tag="scr", bufs=2, padded_shape=[D, 512])
        f_mms = []
        for c in range(3):
            jc = min(D, N - JOFF[c])
            f_mms.append(nc.tensor.matmul(
                fp[0:jc, c * D:(c + 1) * D],
                lhsT=anrm[:, JOFF[c]:JOFF[c] + jc],
                rhs=wout[:],
                start=(c == 0), stop=(c == 2),
                skip_group_check=True,
            ))
        _chain(f_mms)
        fsb = sb.tile([D, 3 * D], F32, tag="fsb", bufs=3)
        cp1 = nc.scalar.copy(fsb[:, 0:2 * D], fp[:, 0:2 * D])
        add_dep_helper(cp1.ins, f_mms[-1].ins, sync=True,
                       reason="read after accumulation group closes")
        cp2 = nc.vector.tensor_copy(fsb[0:87, 2 * D:3 * D], fp[0:87, 2 * D:3 * D])
        add_dep_helper(cp2.ins, f_mms[-1].ins, sync=True,
                       reason="read after accumulation group closes")

        dst01 = out_d[w, 0:256, :].rearrange("(c p) d -> p c d", p=D)
        src01 = fsb[:, 0:256].rearrange("p (c d) -> p c d", c=2)
        nc.sync.dma_start(dst01, src01)
        nc.sync.dma_start(out_d[w, 256:343, :], fsb[0:87, 2 * D:3 * D])

    def proj(w):
        xt = sb.tile([D, NP], BF16, tag="xt", bufs=4)
        nc.sync.dma_start(xt[:], xt_d[w])

        qp = ps.tile([D, N], F32, tag="scr", bufs=2, padded_shape=[D, 512])
        nc.tensor.matmul(qp[:], lhsT=wqkv[:, 0:D], rhs=xt[:, 0:N], start=True, stop=True)
        qsb = sb.tile([D, N], BF16, tag="qsb", bufs=4)
        nc.vector.tensor_copy(qsb[:], qp[:])          # DVE cast

        kp = ps.tile([D, N], F32, tag="scr", bufs=2, padded_shape=[D, 512])
        nc.tensor.matmul(kp[:], lhsT=wqkv[:, D:2 * D], rhs=xt[:, 0:N], start=True, stop=True)
        ksb = sb.tile([D, NP], BF16, tag="ksb", bufs=4)
        if w < 4:
            # pad cols feed sim chunk-2 garbage rows; zero once per buffer slot
            nc.vector.memset(ksb[:, N:NP], 0.0)
        nc.scalar.copy(ksb[:, 0:N], kp[:])            # ACT cast

        vp = ps.tile([D, 3 * D], F32, tag="scr", bufs=2, padded_shape=[D, 512])
        v_mms = []
        for c in range(3):
            v_mms.append(nc.tensor.matmul(
                vp[:, c * D:(c + 1) * D],
                lhsT=xt[:, JOFF[c]:JOFF[c] + D],
                rhs=wqkv[:, 2 * D:3 * D],
                start=(c == 0), stop=(c == 2),
            ))
        _chain(v_mms)
        vsb = sb.tile([D, 3 * D], BF16, tag="vsb", bufs=4)
        cpv = nc.vector.tensor_copy(vsb[:], vp[:])    # DVE cast
        add_dep_helper(cpv.ins, v_mms[-1].ins, sync=True, reason="v accum done")
        return qsb, ksb, vsb

    pending = None
    nxt = proj(0)
    for w in range(NW):
        qsb, ksb, vsb = nxt

        # --- attention accumulators ---
        aop = ps.tile([D, N], F32, tag="ao", bufs=1, padded_shape=[D, 512])
        rsp = ps.tile([D, N], F32, tag="rs", bufs=1, padded_shape=[D, 512])
        ao_mms = []
        rs_mms = []

        def sims_block(c):
            expsim = sb.tile([D, H * N], BF16, tag="es", bufs=4, name=f"es{c}")
            for r in range(2):
                s = ps.tile([D, 2, 512], F32, tag="sim", bufs=2, name=f"s{c}{r}")
                for hh in range(2):
                    h = 2 * r + hh
                    nc.tensor.matmul(
                        s[:, hh, 0:N],
                        lhsT=ksb[DH * h:DH * (h + 1), JOFF[c]:JOFF[c] + D],
                        rhs=qsb[DH * h:DH * (h + 1), 0:N],
                        tile_position=(DH * h, 0),
                        start=True, stop=True,
                    )
                nc.scalar.activation(
                    expsim[:, N * 2 * r:N * (2 * r + 2)],
                    s[:, :, 0:N],
                    mybir.ActivationFunctionType.Exp,
                )
            attn = sb.tile([D, H * N], BF16, tag="attn", bufs=4, name=f"attn{c}")
            nc.vector.tensor_mul(attn[:], expsim[:], eb[:, H * N * c:H * N * (c + 1)])
            return attn

        def av_block(c, attn):
            for h in range(H):
                ao_mms.append(nc.tensor.matmul(
                    aop[DH * h:DH * (h + 1), :],
                    lhsT=vsb[:, D * c + DH * h:D * c + DH * (h + 1)],
                    rhs=attn[:, N * h:N * (h + 1)],
                    tile_position=(0, DH * h),
                    start=(c == 0), stop=(c == 2),
                    skip_group_check=True,
                ))
            for h in range(H):
                rs_mms.append(nc.tensor.matmul(
                    rsp[DH * h:DH * (h + 1), :],
                    lhsT=ones[:, DH * h:DH * (h + 1)],
                    rhs=attn[:, N * h:N * (h + 1)],
                    tile_position=(0, DH * h),
                    start=(c == 0), stop=(c == 2),
                    skip_group_check=True,
                ))

        # one-chunk skew: AV(c) is emitted after sims(c+1), so its attn input
        # is ready by the time the PE reaches it
        attn0 = sims_block(0)
        if pending is not None:
            tail(*pending)
        attn1 = sims_block(1)
        av_block(0, attn0)
        attn2 = sims_block(2)
        av_block(1, attn1)
        av_block(2, attn2)
        if w + 1 < NW:
            nxt = proj(w + 1)
        _chain(ao_mms)
        _chain(rs_mms)

        pending = (w, aop, rsp, rs_mms[-1], ao_mms[-1], None)  # noqa

    tail(*pending)


def _prep_inputs(x, w_qkv, w_out, bias_table, rel_idx):
    x = np.asarray(x, dtype=np.float32)
    w_qkv = np.asarray(w_qkv, dtype=np.float32)
    w_out = np.asarray(w_out, dtype=np.float32)
    bias_table = np.asarray(bias_table, dtype=np.float32)
    rel_idx = np.asarray(rel_idx)

    scale = DH ** -0.5
    wq = w_qkv[:, 0:D] * scale
    wqkv_s = np.concatenate([wq, w_qkv[:, D:3 * D]], axis=1)
    wqkv_bf = wqkv_s.astype(ml_dtypes.bfloat16)
    wout_bf = w_out.astype(ml_dtypes.bfloat16)

    xr = x.reshape(8 * 64, N, D)
    xtf = np.zeros((8 * 64, D, NP), dtype=np.float32)
    xtf[:, :, 0:N] = xr.transpose(0, 2, 1)
    xt = xtf.astype(ml_dtypes.bfloat16).reshape(8, NW, D, NP)

    bias = bias_table[rel_idx]                     # (i, j, h)
    ebT = np.exp(bias).transpose(1, 2, 0)          # (j, h, i)
    tmp = np.zeros((3 * D, H, N), dtype=np.float32)
    tmp[0:N] = ebT
    eb_arr = np.ascontiguousarray(
        tmp.reshape(3, D, H * N).transpose(1, 0, 2).reshape(D, 3 * H * N)
    ).astype(ml_dtypes.bfloat16)

    in_maps = []
    for core in range(8):
        in_maps.append({
            "xt": np.ascontiguousarray(xt[core]),
            "wqkv": wqkv_bf,
            "wout": wout_bf,
            "eb": eb_arr,
        })
    return in_maps


def kernel(x, w_qkv, w_out, bias_table, rel_idx):
    if "nc" not in _cache:
        _cache["nc"] = _build_kernel()
    nc = _cache["nc"]
    in_maps = _prep_inputs(x, w_qkv, w_out, bias_table, rel_idx)
    res = bass_utils.run_bass_kernel_spmd(
        nc, in_maps, core_ids=list(range(8)), trace=TRACE, **TRACE_KWARGS
    )
    _cache["last_result"] = res
    outs = [res.results[c]["out"] for c in range(8)]
    full = np.concatenate(outs, axis=0)             # (512, 343, 128)
    return full.reshape(1, 8, 8, 8, 7, 7, 7, D).astype(np.float32)



# revision 26
# speedup vs baseline: 1.1223x; 1.1223x over previous
"""Trainium2 Bass kernel for windowed 3D attention (nn_Attention_12927851561046).

512 windows of 343-token, 4-head, 32-dim-per-head attention over d=128.
Pure data parallel: 64 windows per core across 8 NeuronCores.

Layout strategy (per window):
  XT (d=128 partitions, 343 tokens free) bf16
  qT/kT = w^T@XT  -> psum -> cast to bf16 sbuf (128=4h*32dh, 343)
  v    = XT^T@wv  -> psum (t-chunks, 128) -> cast bf16 sbuf (128, 3*128)
  simT chunks (j on partitions, i free), 2 heads per psum tile (128, 686)
  exp on ACT (psum->sbuf bf16), *expbias on DVE/GPSIMD (bf16 2x)
  attnout^T + replicated rowsums via ones-matmul (col-tiled, head-packed)
  1/rowsum via DVE reciprocal_approx_fast, normalize+cast on DVE
  final = anrm^T @ w_out -> psum -> copy -> DMA out
"""

import os
import sys
from contextlib import ExitStack

import numpy as np

sys.path.insert(0, "/opt/trn_rl_repo")

import ml_dtypes  # noqa: E402

import concourse.bass as bass  # noqa: E402
import concourse.tile as tile  # noqa: E402
from concourse.tile import add_dep_helper  # noqa: E402
from concourse import bacc, mybir  # noqa: E402
from concourse import bass_utils  # noqa: E402

BF16 = mybir.dt.bfloat16
F32 = mybir.dt.float32

NW = 64          # windows per core
N = 343          # tokens per window
D = 128
H = 4
DH = 32
NP = 384         # padded tokens (zeros beyond 343)
JOFF = [0, 128, 256]

# bisection toggles
NO_GPSIMD = not bool(int(os.environ.get("K_GPSIMD", "0")))
NO_RECIP_FAST = bool(int(os.environ.get("K_NO_RECIP_FAST", "0")))
NO_TILEPOS_SIM = bool(int(os.environ.get("K_NO_TILEPOS_SIM", "0")))
NO_TILEPOS_AO = bool(int(os.environ.get("K_NO_TILEPOS_AO", "0")))
STAGE = int(os.environ.get("K_STAGE", "0"))  # 0=full, 1..4 truncation
H2ONLY = bool(int(os.environ.get("K_H2ONLY", "0")))  # heads use bases {0,32} only (wrong data, mechanism test)
EXP_SPLIT = bool(int(os.environ.get("K_EXP_SPLIT", "1")))  # per-bank exp reads

# module-level knobs (test.py pokes these)
TRACE = False
TRACE_KWARGS = {}

_cache = {}


def _build_kernel():
    nc = bacc.Bacc(
        "TRN2",
        target_bir_lowering=False,
        debug=False,
        enable_asserts=False,
        num_devices=8,
    )
    xt_d = nc.dram_tensor("xt", (NW, D, NP), BF16, kind="ExternalInput").ap()
    wqkv_d = nc.dram_tensor("wqkv", (D, 3 * D), BF16, kind="ExternalInput").ap()
    wout_d = nc.dram_tensor("wout", (D, D), BF16, kind="ExternalInput").ap()
    eb_d = nc.dram_tensor("eb", (D, 3 * H * N), BF16, kind="ExternalInput").ap()
    out_d = nc.dram_tensor("out", (NW, N, D), F32, kind="ExternalOutput").ap()

    with tile.TileContext(nc) as tc:
        with ExitStack() as ctx:
            _body(ctx, tc, out_d, xt_d, wqkv_d, wout_d, eb_d)

    nc.compile()
    return nc


def _chain(insts):
    for a, b in zip(insts[1:], insts[:-1]):
        add_dep_helper(a.ins, b.ins, sync=False, reason="psum accumulation order")


def _body(ctx, tc, out_d, xt_d, wqkv_d, wout_d, eb_d):
    nc = tc.nc

    const = ctx.enter_context(tc.tile_pool(name="const", bufs=1))
    sb = ctx.enter_context(tc.tile_pool(name="sb", bufs=2))
    ps = ctx.enter_context(tc.tile_pool(name="ps", bufs=1, space="PSUM"))

    # constants
    wqkv = const.tile([D, 3 * D], BF16)
    nc.sync.dma_start(wqkv[:], wqkv_d[:])
    wout = const.tile([D, D], BF16)
    nc.sync.dma_start(wout[:], wout_d[:])
    eb = const.tile([D, 3 * H * N], BF16)
    nc.sync.dma_start(eb[:], eb_d[:])
    ones = const.tile([D, D], BF16)
    nc.vector.memset(ones[:], 1.0)

    def tail(w, aop, rsp, rs_last, ao_last, anrm_of):
        """Deferred per-window tail: normalize, out-projection, output DMA."""
        recip = sb.tile([D, N], F32, tag="recip", bufs=3)
        rc = nc.vector.reciprocal_approx_fast(recip[:], rsp[:])
        add_dep_helper(rc.ins, rs_last.ins, sync=True,
                       reason="read rowsums after accumulation closes")
        anrm = sb.tile([D, N], BF16, tag="anrm", bufs=3)
        tt = nc.vector.tensor_mul(anrm[:], aop[:], recip[:])
        add_dep_helper(tt.ins, ao_last.ins, sync=True,
                       reason="read ao after accumulation closes")

        fp = ps.tile([D, 3 * D], F32, tag="v", bufs=1, padded_shape=[D, 512])
        f_mms = []
        for c in range(3):
            jc = min(D, N - JOFF[c])
            f_mms.append(nc.tensor.matmul(
                fp[0:jc, c * D:(c + 1) * D],
                lhsT=anrm[:, JOFF[c]:JOFF[c] + jc],
                rhs=wout[:],
                start=(c == 0), stop=(c == 2),
                skip_group_check=True,
            ))
        _chain(f_mms)
        fsb = sb.tile([D, 3 * D], F32, tag="fsb", bufs=3)
        cp1 = nc.scalar.copy(fsb[:, 0:2 * D], fp[:, 0:2 * D])
        add_dep_helper(cp1.ins, f_mms[-1].ins, sync=True,
                       reason="read after accumulation group closes")
        cp2 = nc.vector.tensor_copy(fsb[0:87, 2 * D:3 * D], fp[0:87, 2 * D:3 * D])
        add_dep_helper(cp2.ins, f_mms[-1].ins, sync=True,
                       reason="read after accumulation group closes")

        dst01 = out_d[w, 0:256, :].rearrange("(c p) d -> p c d", p=D)
        src01 = fsb[:, 0:256].rearrange("p (c d) -> p c d", c=2)
        nc.sync.dma_start(dst01, src01)
        nc.sync.dma_start(out_d[w, 256:343, :], fsb[0:87, 2 * D:3 * D])

    def proj(w):
        xt = sb.tile([D, NP], BF16, tag="xt", bufs=4)
        nc.sync.dma_start(xt[:], xt_d[w])

        qp = ps.tile([D, N], F32, tag="qk", bufs=1, padded_shape=[D, 512])
        nc.tensor.matmul(qp[:], lhsT=wqkv[:, 0:D], rhs=xt[:, 0:N], start=True, stop=True)
        qsb = sb.tile([D, N], BF16, tag="qsb", bufs=4)
        nc.vector.tensor_copy(qsb[:], qp[:])          # DVE cast

        kp = ps.tile([D, N], F32, tag="qk", bufs=1, padded_shape=[D, 512])
        nc.tensor.matmul(kp[:], lhsT=wqkv[:, D:2 * D], rhs=xt[:, 0:N], start=True, stop=True)
        ksb = sb.tile([D, NP], BF16, tag="ksb", bufs=4)
        if w < 4:
            # pad cols feed sim chunk-2 garbage rows; zero once per buffer slot
            nc.vector.memset(ksb[:, N:NP], 0.0)
        nc.scalar.copy(ksb[:, 0:N], kp[:])            # ACT cast

        vp = ps.tile([D, 3 * D], F32, tag="v", bufs=1, padded_shape=[D, 512])
        v_mms = []
        for c in range(3):
            v_mms.append(nc.tensor.matmul(
                vp[:, c * D:(c + 1) * D],
                lhsT=xt[:, JOFF[c]:JOFF[c] + D],
                rhs=wqkv[:, 2 * D:3 * D],
                start=(c == 0), stop=(c == 2),
            ))
        _chain(v_mms)
        vsb = sb.tile([D, 3 * D], BF16, tag="vsb", bufs=4)
        cpv = nc.vector.tensor_copy(vsb[:], vp[:])    # DVE cast
        add_dep_helper(cpv.ins, v_mms[-1].ins, sync=True, reason="v accum done")
        return qsb, ksb, vsb

    pending = None
    nxt = proj(0)
    for w in range(NW):
        qsb, ksb, vsb = nxt

        # --- attention accumulators ---
        aop = ps.tile([D, N], F32, tag="ao", bufs=1, padded_shape=[D, 512])
        rsp = ps.tile([D, N], F32, tag="rs", bufs=1, padded_shape=[D, 512])
        ao_mms = []
        rs_mms = []

        def sims_block(c):
            expsim = sb.tile([D, H * N], BF16, tag="es", bufs=4, name=f"es{c}")
            for r in range(2):
                s = ps.tile([D, 2, 512], F32, tag="sim", bufs=2, name=f"s{c}{r}")
                for hh in range(2):
                    h = 2 * r + hh
                    nc.tensor.matmul(
                        s[:, hh, 0:N],
                        lhsT=ksb[DH * h:DH * (h + 1), JOFF[c]:JOFF[c] + D],
                        rhs=qsb[DH * h:DH * (h + 1), 0:N],
                        tile_position=(DH * h, 0),
                        start=True, stop=True,
                    )
                nc.scalar.activation(
                    expsim[:, N * 2 * r:N * (2 * r + 2)],
                    s[:, :, 0:N],
                    mybir.ActivationFunctionType.Exp,
                )
            attn = sb.tile([D, H * N], BF16, tag="attn", bufs=4, name=f"attn{c}")
            nc.vector.tensor_mul(attn[:], expsim[:], eb[:, H * N * c:H * N * (c + 1)])
            return attn

        def av_block(c, attn):
            for h in range(H):
                ao_mms.append(nc.tensor.matmul(
                    aop[DH * h:DH * (h + 1), :],
                    lhsT=vsb[:, D * c + DH * h:D * c + DH * (h + 1)],
                    rhs=attn[:, N * h:N * (h + 1)],
                    tile_position=(0, DH * h),
                    start=(c == 0), stop=(c == 2),
                    skip_group_check=True,
                ))
            for h in range(H):
                rs_mms.append(nc.tensor.matmul(
                    rsp[DH * h:DH * (h + 1), :],
                    lhsT=ones[:, DH * h:DH * (h + 1)],
                    rhs=attn[:, N * h:N * (h + 1)],
                    tile_position=(0, DH * h),
                    start=(c == 0), stop=(c == 2),
                    skip_group_check=True,
                ))

        # one-chunk skew: AV(c) is emitted after sims(c+1), so its attn input
        # is ready by the time the PE reaches it
        attn0 = sims_block(0)
        if pending is not None:
            tail(*pending)
        attn1 = sims_block(1)
        attn2 = sims_block(2)
        av_block(0, attn0)
        av_block(1, attn1)
        av_block(2, attn2)
        if w + 1 < NW:
            nxt = proj(w + 1)
        _chain(ao_mms)
        _chain(rs_mms)

        pending = (w, aop, rsp, rs_mms[-1], ao_mms[-1], None)  # noqa

    tail(*pending)


def _prep_inputs(x, w_qkv, w_out, bias_table, rel_idx):
    x = np.asarray(x, dtype=np.float32)
    w_qkv = np.asarray(w_qkv, dtype=np.float32)
    w_out = np.asarray(w_out, dtype=np.float32)
    bias_table = np.asarray(bias_table, dtype=np.float32)
    rel_idx = np.asarray(rel_idx)

    scale = DH ** -0.5
    wq = w_qkv[:, 0:D] * scale
    wqkv_s = np.concatenate([wq, w_qkv[:, D:3 * D]], axis=1)
    wqkv_bf = wqkv_s.astype(ml_dtypes.bfloat16)
    wout_bf = w_out.astype(ml_dtypes.bfloat16)

    xr = x.reshape(8 * 64, N, D)
    xtf = np.zeros((8 * 64, D, NP), dtype=np.float32)
    xtf[:, :, 0:N] = xr.transpose(0, 2, 1)
    xt = xtf.astype(ml_dtypes.bfloat16).reshape(8, NW, D, NP)

    bias = bias_table[rel_idx]                     # (i, j, h)
    ebT = np.exp(bias).transpose(1, 2, 0)          # (j, h, i)
    tmp = np.zeros((3 * D, H, N), dtype=np.float32)
    tmp[0:N] = ebT
    eb_arr = np.ascontiguousarray(
        tmp.reshape(3, D, H * N).transpose(1, 0, 2).reshape(D, 3 * H * N)
    ).astype(ml_dtypes.bfloat16)

    in_maps = []
    for core in range(8):
        in_maps.append({
            "xt": np.ascontiguousarray(xt[core]),
            "wqkv": wqkv_bf,
            "wout": wout_bf,
            "eb": eb_arr,
        })
    return in_maps


def kernel(x, w_qkv, w_out, bias_table, rel_idx):
    if "nc" not in _cache:
        _cache["nc"] = _build_kernel()
    nc = _cache["nc"]
    in_maps = _prep_inputs(x, w_qkv, w_out, bias_table, rel_idx)
    res = bass_utils.run_bass_kernel_spmd(
        nc, in_maps, core_ids=list(range(8)), trace=TRACE, **TRACE_KWARGS
    )
    _cache["last_result"] = res
    outs = [res.results[c]["out"] for c in range(8)]
    full = np.concatenate(outs, axis=0)             # (512, 343, 128)
    return full.reshape(1, 8, 8, 8, 7, 7, 7, D).astype(np.float32)



# revision 27
# speedup vs baseline: 1.1435x; 1.0189x over previous
"""Trainium2 Bass kernel for windowed 3D attention (nn_Attention_12927851561046).

512 windows of 343-token, 4-head, 32-dim-per-head attention over d=128.
Pure data parallel: 64 windows per core across 8 NeuronCores.

Layout strategy (per window):
  XT (d=128 partitions, 343 tokens free) bf16
  qT/kT = w^T@XT  -> psum -> cast to bf16 sbuf (128=4h*32dh, 343)
  v    = XT^T@wv  -> psum (t-chunks, 128) -> cast bf16 sbuf (128, 3*128)
  simT chunks (j on partitions, i free), 2 heads per psum tile (128, 686)
  exp on ACT (psum->sbuf bf16), *expbias on DVE/GPSIMD (bf16 2x)
  attnout^T + replicated rowsums via ones-matmul (col-tiled, head-packed)
  1/rowsum via DVE reciprocal_approx_fast, normalize+cast on DVE
  final = anrm^T @ w_out -> psum -> copy -> DMA out
"""

import os
import sys
from contextlib import ExitStack

import numpy as np

sys.path.insert(0, "/opt/trn_rl_repo")

import ml_dtypes  # noqa: E402

import concourse.bass as bass  # noqa: E402
import concourse.tile as tile  # noqa: E402
from concourse.tile import add_dep_helper  # noqa: E402
from concourse import bacc, mybir  # noqa: E402
from concourse import bass_utils  # noqa: E402

BF16 = mybir.dt.bfloat16
F32 = mybir.dt.float32

NW = 64          # windows per core
N = 343          # tokens per window
D = 128
H = 4
DH = 32
NP = 384         # padded tokens (zeros beyond 343)
JOFF = [0, 128, 256]

# bisection toggles
NO_GPSIMD = not bool(int(os.environ.get("K_GPSIMD", "0")))
NO_RECIP_FAST = bool(int(os.environ.get("K_NO_RECIP_FAST", "0")))
NO_TILEPOS_SIM = bool(int(os.environ.get("K_NO_TILEPOS_SIM", "0")))
NO_TILEPOS_AO = bool(int(os.environ.get("K_NO_TILEPOS_AO", "0")))
STAGE = int(os.environ.get("K_STAGE", "0"))  # 0=full, 1..4 truncation
H2ONLY = bool(int(os.environ.get("K_H2ONLY", "0")))  # heads use bases {0,32} only (wrong data, mechanism test)
EXP_SPLIT = bool(int(os.environ.get("K_EXP_SPLIT", "1")))  # per-bank exp reads

# module-level knobs (test.py pokes these)
TRACE = False
TRACE_KWARGS = {}

_cache = {}


def _build_kernel():
    nc = bacc.Bacc(
        "TRN2",
        target_bir_lowering=False,
        debug=False,
        enable_asserts=False,
        num_devices=8,
    )
    xt_d = nc.dram_tensor("xt", (NW, D, NP), BF16, kind="ExternalInput").ap()
    wqkv_d = nc.dram_tensor("wqkv", (D, 3 * D), BF16, kind="ExternalInput").ap()
    wout_d = nc.dram_tensor("wout", (D, D), BF16, kind="ExternalInput").ap()
    eb_d = nc.dram_tensor("eb", (D, 3 * H * N), BF16, kind="ExternalInput").ap()
    out_d = nc.dram_tensor("out", (NW, N, D), F32, kind="ExternalOutput").ap()

    with tile.TileContext(nc) as tc:
        with ExitStack() as ctx:
            _body(ctx, tc, out_d, xt_d, wqkv_d, wout_d, eb_d)

    nc.compile()
    return nc


def _chain(insts):
    for a, b in zip(insts[1:], insts[:-1]):
        add_dep_helper(a.ins, b.ins, sync=False, reason="psum accumulation order")


def _body(ctx, tc, out_d, xt_d, wqkv_d, wout_d, eb_d):
    nc = tc.nc

    const = ctx.enter_context(tc.tile_pool(name="const", bufs=1))
    sb = ctx.enter_context(tc.tile_pool(name="sb", bufs=2))
    ps = ctx.enter_context(tc.tile_pool(name="ps", bufs=1, space="PSUM"))

    # constants
    wqkv = const.tile([D, 3 * D], BF16)
    nc.sync.dma_start(wqkv[:], wqkv_d[:])
    wout = const.tile([D, D], BF16)
    nc.sync.dma_start(wout[:], wout_d[:])
    eb = const.tile([D, 3 * H * N], BF16)
    nc.sync.dma_start(eb[:], eb_d[:])
    ones = const.tile([D, D], BF16)
    nc.vector.memset(ones[:], 1.0)

    def tail(w, aop, rsp, rs_last, ao_last, anrm_of):
        """Deferred per-window tail: normalize, out-projection, output DMA."""
        recip = sb.tile([D, N], F32, tag="recip", bufs=3)
        rc = nc.vector.reciprocal_approx_fast(recip[:], rsp[:])
        add_dep_helper(rc.ins, rs_last.ins, sync=True,
                       reason="read rowsums after accumulation closes")
        anrm = sb.tile([D, N], BF16, tag="anrm", bufs=3)
        tt = nc.vector.tensor_mul(anrm[:], aop[:], recip[:])
        add_dep_helper(tt.ins, ao_last.ins, sync=True,
                       reason="read ao after accumulation closes")

        fp = ps.tile([D, 3 * D], F32, tag="v", bufs=1, padded_shape=[D, 512])
        f_mms = []
        for c in range(3):
            jc = min(D, N - JOFF[c])
            f_mms.append(nc.tensor.matmul(
                fp[0:jc, c * D:(c + 1) * D],
                lhsT=anrm[:, JOFF[c]:JOFF[c] + jc],
                rhs=wout[:],
                start=(c == 0), stop=(c == 2),
                skip_group_check=True,
            ))
        _chain(f_mms)
        fsb = sb.tile([D, 3 * D], F32, tag="fsb", bufs=3)
        cp1 = nc.scalar.copy(fsb[:, 0:2 * D], fp[:, 0:2 * D])
        add_dep_helper(cp1.ins, f_mms[-1].ins, sync=True,
                       reason="read after accumulation group closes")
        cp2 = nc.vector.tensor_copy(fsb[0:87, 2 * D:3 * D], fp[0:87, 2 * D:3 * D])
        add_dep_helper(cp2.ins, f_mms[-1].ins, sync=True,
                       reason="read after accumulation group closes")

        dst01 = out_d[w, 0:256, :].rearrange("(c p) d -> p c d", p=D)
        src01 = fsb[:, 0:256].rearrange("p (c d) -> p c d", c=2)
        nc.sync.dma_start(dst01, src01)
        nc.sync.dma_start(out_d[w, 256:343, :], fsb[0:87, 2 * D:3 * D])

    def proj(w):
        xt = sb.tile([D, NP], BF16, tag="xt", bufs=4)
        nc.sync.dma_start(xt[:], xt_d[w])

        qp = ps.tile([D, N], F32, tag="qk", bufs=1, padded_shape=[D, 512])
        nc.tensor.matmul(qp[:], lhsT=wqkv[:, 0:D], rhs=xt[:, 0:N], start=True, stop=True)
        qsb = sb.tile([D, N], BF16, tag="qsb", bufs=4)
        nc.vector.tensor_copy(qsb[:], qp[:])          # DVE cast

        kp = ps.tile([D, N], F32, tag="qk", bufs=1, padded_shape=[D, 512])
        nc.tensor.matmul(kp[:], lhsT=wqkv[:, D:2 * D], rhs=xt[:, 0:N], start=True, stop=True)
        ksb = sb.tile([D, NP], BF16, tag="ksb", bufs=4)
        if w < 4:
            # pad cols feed sim chunk-2 garbage rows; zero once per buffer slot
            nc.vector.memset(ksb[:, N:NP], 0.0)
        nc.scalar.copy(ksb[:, 0:N], kp[:])            # ACT cast

        vp = ps.tile([D, 3 * D], F32, tag="v", bufs=1, padded_shape=[D, 512])
        v_mms = []
        for c in range(3):
            v_mms.append(nc.tensor.matmul(
                vp[:, c * D:(c + 1) * D],
                lhsT=xt[:, JOFF[c]:JOFF[c] + D],
                rhs=wqkv[:, 2 * D:3 * D],
                start=(c == 0), stop=(c == 2),
            ))
        _chain(v_mms)
        vsb = sb.tile([D, 3 * D], BF16, tag="vsb", bufs=4)
        cpv = nc.vector.tensor_copy(vsb[:], vp[:])    # DVE cast
        add_dep_helper(cpv.ins, v_mms[-1].ins, sync=True, reason="v accum done")
        return qsb, ksb, vsb

    pending = None
    nxt = proj(0)
    for w in range(NW):
        qsb, ksb, vsb = nxt

        # --- attention accumulators ---
        aop = ps.tile([D, N], F32, tag="ao", bufs=1, padded_shape=[D, 512])
        rsp = ps.tile([D, N], F32, tag="rs", bufs=1, padded_shape=[D, 512])
        ao_mms = []
        rs_mms = []

        def sims_block(c, mid=None):
            expsim = sb.tile([D, H * N], BF16, tag="es", bufs=4, name=f"es{c}")
            for r in range(2):
                s = ps.tile([D, 2, 512], F32, tag="sim", bufs=2, name=f"s{c}{r}")
                for hh in range(2):
                    h = 2 * r + hh
                    nc.tensor.matmul(
                        s[:, hh, 0:N],
                        lhsT=ksb[DH * h:DH * (h + 1), JOFF[c]:JOFF[c] + D],
                        rhs=qsb[DH * h:DH * (h + 1), 0:N],
                        tile_position=(DH * h, 0),
                        start=True, stop=True,
                    )
                nc.scalar.activation(
                    expsim[:, N * 2 * r:N * (2 * r + 2)],
                    s[:, :, 0:N],
                    mybir.ActivationFunctionType.Exp,
                )
                if r == 0 and mid is not None:
                    mid()
            attn = sb.tile([D, H * N], BF16, tag="attn", bufs=4, name=f"attn{c}")
            nc.vector.tensor_mul(attn[:], expsim[:], eb[:, H * N * c:H * N * (c + 1)])
            return attn

        def av_block(c, attn):
            for h in range(H):
                ao_mms.append(nc.tensor.matmul(
                    aop[DH * h:DH * (h + 1), :],
                    lhsT=vsb[:, D * c + DH * h:D * c + DH * (h + 1)],
                    rhs=attn[:, N * h:N * (h + 1)],
                    tile_position=(0, DH * h),
                    start=(c == 0), stop=(c == 2),
                    skip_group_check=True,
                ))
            for h in range(H):
                rs_mms.append(nc.tensor.matmul(
                    rsp[DH * h:DH * (h + 1), :],
                    lhsT=ones[:, DH * h:DH * (h + 1)],
                    rhs=attn[:, N * h:N * (h + 1)],
                    tile_position=(0, DH * h),
                    start=(c == 0), stop=(c == 2),
                    skip_group_check=True,
                ))

        # one-chunk skew: AV(c) is emitted after sims(c+1), so its attn input
        # is ready by the time the PE reaches it
        pw = pending
        attn0 = sims_block(0, mid=(lambda: tail(*pw)) if pw is not None else None)
        attn1 = sims_block(1)
        av_block(0, attn0)
        attn2 = sims_block(2)
        av_block(1, attn1)
        av_block(2, attn2)
        if w + 1 < NW:
            nxt = proj(w + 1)
        _chain(ao_mms)
        _chain(rs_mms)

        pending = (w, aop, rsp, rs_mms[-1], ao_mms[-1], None)  # noqa

    tail(*pending)


def _prep_inputs(x, w_qkv, w_out, bias_table, rel_idx):
    x = np.asarray(x, dtype=np.float32)
    w_qkv = np.asarray(w_qkv, dtype=np.float32)
    w_out = np.asarray(w_out, dtype=np.float32)
    bias_table = np.asarray(bias_table, dtype=np.float32)
    rel_idx = np.asarray(rel_idx)

    scale = DH ** -0.5
    wq = w_qkv[:, 0:D] * scale
    wqkv_s = np.concatenate([wq, w_qkv[:, D:3 * D]], axis=1)
    wqkv_bf = wqkv_s.astype(ml_dtypes.bfloat16)
    wout_bf = w_out.astype(ml_dtypes.bfloat16)

    xr = x.reshape(8 * 64, N, D)
    xtf = np.zeros((8 * 64, D, NP), dtype=np.float32)
    xtf[:, :, 0:N] = xr.transpose(0, 2, 1)
    xt = xtf.astype(ml_dtypes.bfloat16).reshape(8, NW, D, NP)

    bias = bias_table[rel_idx]                     # (i, j, h)
    ebT = np.exp(bias).transpose(1, 2, 0)          # (j, h, i)
    tmp = np.zeros((3 * D, H, N), dtype=np.float32)
    tmp[0:N] = ebT
    eb_arr = np.ascontiguousarray(
        tmp.reshape(3, D, H * N).transpose(1, 0, 2).reshape(D, 3 * H * N)
    ).astype(ml_dtypes.bfloat16)

    in_maps = []
    for core in range(8):
        in_maps.append({
            "xt": np.ascontiguousarray(xt[core]),
            "wqkv": wqkv_bf,
            "wout": wout_bf,
            "eb": eb_arr,
        })
    return in_maps


def kernel(x, w_qkv, w_out, bias_table, rel_idx):
    if "nc" not in _cache:
        _cache["nc"] = _build_kernel()
    nc = _cache["nc"]
    in_maps = _prep_inputs(x, w_qkv, w_out, bias_table, rel_idx)
    res = bass_utils.run_bass_kernel_spmd(
        nc, in_maps, core_ids=list(range(8)), trace=TRACE, **TRACE_KWARGS
    )
    _cache["last_result"] = res
    outs = [res.results[c]["out"] for c in range(8)]
    full = np.concatenate(outs, axis=0)             # (512, 343, 128)
    return full.reshape(1, 8, 8, 8, 7, 7, 7, D).astype(np.float32)

